# revision 1
# baseline (speedup 1.0000x reference)
"""Trainium2 Bass kernel for nn_EquivariantLayer (gnn_message_passing).

Computes, per batch element:  out = x @ A - ones(N,1) @ (colsum(x) @ B)
with x [65536, 64] f32, A/B [64, 64] f32.

Sharding: batch axis (8) -> 8 NeuronCores, A/B replicated; no collectives.

Per-core dataflow (two phases; input and output DMA cannot overlap because
every output row depends on colsum(x)):
  Phase 1 (streaming over 16 tiles of 4096 rows):
    - HWDGE DMA x tile (1 MiB, contiguous) into SBUF fp32
    - GPSIMD cast fp32 -> bf16
    - DVE strided reduce (fp32, exact-ish) accumulating per-c column sums
    - PE transposes [128,64] blocks -> [64,128] bf16 into PSUM
    - ACT evicts transposed blocks into resident bf16 x^T tiles (8 MiB)
  Interlude: finalize s = colsum(x) (fp32), bc = ones (x) (s @ B) broadcast tile
  Phase 2 (per tile):
    - PE matmuls, x^T block stationary [64,128], A (bf16) moving -> natural
      [128,64] fp32 PSUM blocks (row-tiled pairs over PE row groups 0/64)
    - DVE fused (psum - bc) evict into out staging
    - HWDGE DMA out tile (1 MiB, contiguous)

Precision: s/B path entirely fp32; only x@A runs in bf16.  The output norm is
dominated by the (s@B) term, so overall rel err ~2e-4.
"""

import sys

for _p in ("/opt/trn_rl_repo",):
    if _p not in sys.path:
        sys.path.insert(0, _p)

import numpy as np

import concourse.bass as bass
import concourse.tile as tile
from concourse import bacc, mybir
from concourse.masks import make_identity

F32 = mybir.dt.float32
BF16 = mybir.dt.bfloat16

N_CORES = 8
N_ROWS = 65536
C = 64
P = 128


def build(n_rows=N_ROWS, tile_rows=4096):
    """Build the single-core Tile program (SPMD across cores via inputs)."""
    assert n_rows % tile_rows == 0
    nt = n_rows // tile_rows          # number of big tiles
    jb = tile_rows // P               # 64-col blocks per tile (rows per partition)
    free_f32 = tile_rows * C // P     # f32 elems per partition per tile
    assert jb % 16 == 0
    hb = jb // 16                     # transpose psum banks per tile

    nc = bacc.Bacc(
        "TRN2", target_bir_lowering=False, debug=False, num_devices=N_CORES
    )
    x_d = nc.dram_tensor("x", [n_rows, C], F32, kind="ExternalInput").ap()
    a_d = nc.dram_tensor("A", [C, C], F32, kind="ExternalInput").ap()
    b_d = nc.dram_tensor("B", [C, C], F32, kind="ExternalInput").ap()
    o_d = nc.dram_tensor("out", [n_rows, C], F32, kind="ExternalOutput").ap()

    with tile.TileContext(nc) as tc:
        with (
            tc.tile_pool(name="consts", bufs=1) as consts,
            tc.tile_pool(name="xin", bufs=3) as xin,
            tc.tile_pool(name="xbfp", bufs=2) as xbfp,
            tc.tile_pool(name="xtp", bufs=2 * nt) as xtp,
            tc.tile_pool(name="outp", bufs=3) as outp,
            tc.tile_pool(name="statsp", bufs=1) as statsp,
            tc.tile_pool(name="tpsum", bufs=2, space="PSUM") as tpsum,
            tc.tile_pool(name="opsum", bufs=4, space="PSUM") as opsum,
            tc.tile_pool(name="spsum", bufs=1, space="PSUM") as spsum,
        ):
            # ---- constants ----
            ident = consts.tile([P, P], BF16)
            make_identity(nc, ident[:])
            a_f32 = consts.tile([P, C], F32)
            nc.gpsimd.dma_start(out=a_f32[0:64, :], in_=a_d)
            nc.gpsimd.dma_start(out=a_f32[64:128, :], in_=a_d)
            a_bf = consts.tile([P, C], BF16)
            nc.gpsimd.tensor_copy(out=a_bf[:], in_=a_f32[:])
            b_sb = consts.tile([64, C], F32)
            nc.gpsimd.dma_start(out=b_sb[:], in_=b_d)
            ones_p = consts.tile([P, 1], F32)
            nc.vector.memset(ones_p[:], 1.0)
            ones_m = consts.tile([64, P], F32)
            nc.vector.memset(ones_m[:], 1.0)

            stats = statsp.tile([P, nt * C], F32)

            # ---- phase 1: load + cast + reduce + transpose ----
            xts = []  # [tile][h] -> resident bf16 x^T tiles [128, 1024]
            for t in range(nt):
                xf = xin.tile([P, free_f32], F32)
                nc.sync.dma_start(
                    out=xf[:],
                    in_=x_d[t * tile_rows : (t + 1) * tile_rows, :].rearrange(
                        "(p j) c -> p (j c)", p=P
                    ),
                )
                xb = xbfp.tile([P, free_f32], BF16)
                nc.gpsimd.tensor_copy(out=xb[:], in_=xf[:])
                nc.vector.reduce_sum(
                    out=stats[:, t * C : (t + 1) * C],
                    in_=xf[:].rearrange("p (j c) -> p c j", j=jb),
                    axis=mybir.AxisListType.X,
                )
                per_tile = []
                for h in range(hb):
                    tb = tpsum.tile([P, 1024], BF16, tag="tb")
                    for jj in range(16):
                        j = 16 * h + jj
                        colg = 0 if jj < 8 else 64
                        col = 128 * (jj % 8)
                        nc.tensor.transpose(
                            out=tb[colg : colg + 64, col : col + 128],
                            in_=xb[:, j * C : (j + 1) * C],
                            identity=ident[:],
                            tile_position=(0, colg),
                        )
                    xt_sb = xtp.tile([P, 1024], BF16, tag="xt")
                    nc.scalar.copy(out=xt_sb[:], in_=tb[:])
                    per_tile.append(xt_sb)
                xts.append(per_tile)

            # ---- interlude: finalize s and the broadcast tile bc ----
            part2 = statsp.tile([P, C], F32)
            nc.vector.reduce_sum(
                out=part2[:],
                in_=stats[:].rearrange("p (t c) -> p c t", t=nt),
                axis=mybir.AxisListType.X,
            )
            st_ps = spsum.tile([64, 1], F32, tag="sps")
            nc.tensor.matmul(
                out=st_ps[:], lhsT=part2[:], rhs=ones_p[:], start=True, stop=True
            )
            st_sb = consts.tile([64, 1], F32)
            nc.scalar.copy(out=st_sb[:], in_=st_ps[:])
            bs_sb = consts.tile([64, C], F32)
            nc.vector.tensor_scalar_mul(out=bs_sb[:], in0=b_sb[:], scalar1=st_sb[:])
            bc_ps = spsum.tile([P, C], F32, tag="sps")
            nc.tensor.matmul(
                out=bc_ps[:], lhsT=ones_m[:], rhs=bs_sb[:], start=True, stop=True
            )
            bc_sb = consts.tile([P, C], F32)
            nc.scalar.copy(out=bc_sb[:], in_=bc_ps[:])

            bc_ap = bc_sb[:]
            bc_bcast = bass.AP(
                tensor=bc_ap.tensor,
                offset=bc_ap.offset,
                ap=[list(bc_ap.ap[0]), [0, 8], list(bc_ap.ap[1])],
            )

            # ---- phase 2: matmuls + subtract-evict + store ----
            for t in range(nt):
                ot = outp.tile([P, free_f32], F32)
                for h in range(hb):
                    xt_sb = xts[t][h]
                    oa = opsum.tile([P, 512], F32, tag="ob")
                    ob = opsum.tile([P, 512], F32, tag="ob")
                    for jj in range(8):
                        col = 128 * jj
                        nc.tensor.matmul(
                            out=oa[:, 64 * jj : 64 * jj + 64],
                            lhsT=xt_sb[0:64, col : col + 128],
                            rhs=a_bf[0:64, :],
                            start=True,
                            stop=True,
                            tile_position=(0, 0),
                        )
                        nc.tensor.matmul(
                            out=ob[:, 64 * jj : 64 * jj + 64],
                            lhsT=xt_sb[64:128, col : col + 128],
                            rhs=a_bf[64:128, :],
                            start=True,
                            stop=True,
                            tile_position=(64, 0),
                        )
                    seg = 1024 * h
                    nc.vector.tensor_sub(
                        out=ot[:, seg : seg + 512].rearrange(
                            "p (j c) -> p j c", c=C
                        ),
                        in0=oa[:].rearrange("p (j c) -> p j c", c=C),
                        in1=bc_bcast,
                    )
                    nc.vector.tensor_sub(
                        out=ot[:, seg + 512 : seg + 1024].rearrange(
                            "p (j c) -> p j c", c=C
                        ),
                        in0=ob[:].rearrange("p (j c) -> p j c", c=C),
                        in1=bc_bcast,
                    )
                nc.sync.dma_start(
                    out=o_d[t * tile_rows : (t + 1) * tile_rows, :].rearrange(
                        "(p j) c -> p (j c)", p=P
                    ),
                    in_=ot[:],
                )

    nc.compile()
    return nc


_CACHE = {}


def _get_compiled():
    if "nc" not in _CACHE:
        _CACHE["nc"] = build()
    return _CACHE["nc"]


def _run(nc, x, A, B, **kwargs):
    from concourse.bass_utils import run_bass_kernel_spmd

    x = np.ascontiguousarray(np.asarray(x, dtype=np.float32))
    A = np.ascontiguousarray(np.asarray(A, dtype=np.float32))
    B = np.ascontiguousarray(np.asarray(B, dtype=np.float32))
    n_cores = x.shape[0]
    in_maps = [{"x": x[i], "A": A, "B": B} for i in range(n_cores)]
    res = run_bass_kernel_spmd(nc, in_maps, core_ids=list(range(n_cores)), **kwargs)
    out = np.stack([res.results[i]["out"] for i in range(n_cores)], axis=0)
    return out, res


def kernel(x, A, B):
    nc = _get_compiled()
    out, _ = _run(nc, x, A, B)
    return out.astype(np.float32)


# revision 4
# speedup vs baseline: 1.1911x; 1.1911x over previous
"""Trainium2 Bass kernel for nn_EquivariantLayer (gnn_message_passing).

Computes, per batch element:  out = x @ A - ones(N,1) @ (colsum(x) @ B)
with x [65536, 64] f32, A/B [64, 64] f32.

Sharding: batch axis (8) -> 8 NeuronCores, A/B replicated; no collectives.

Per-core dataflow (two phases; input and output DMA cannot overlap because
every output row depends on colsum(x)):
  Phase 1 (streaming tiles):
    - HWDGE DMA x tile (contiguous) into SBUF fp32
    - DVE cast fp32 -> bf16; DVE folding adds accumulate fp32 column sums
    - PE pair-transposes [128,128] bf16 blocks into PSUM
    - ACT evicts transposed blocks into resident bf16 x^T tiles (8 MiB)
  Interlude: s = colsum(x) (fp32); -s@B split into bf16 hi+lo rows
  Phase 2 (per tile):
    - PE matmuls: x^T pair block [128,128] stationary, block-diag [[A,0],[0,A]]
      bf16 moving -> natural [128,128] fp32 PSUM (start=True, stop=False)
    - PE K=2 ones-matmul accumulates -(s@B) (hi+lo bf16) onto each PSUM bank
    - ACT plain-copy evicts bank -> out staging
    - HWDGE DMA out tile (contiguous)

Precision: s path and -s@B fully fp32 (hi/lo bf16 split is exact to ~2^-17);
only x@A runs in bf16.  Output norm is dominated by the s@B term, so overall
rel err ~1e-4.
"""

import sys

for _p in ("/opt/trn_rl_repo",):
    if _p not in sys.path:
        sys.path.insert(0, _p)

import numpy as np

import concourse.bass as bass
import concourse.tile as tile
from concourse import bacc, mybir
from concourse.masks import make_identity

F32 = mybir.dt.float32
BF16 = mybir.dt.bfloat16

N_CORES = 8
N_ROWS = 65536
C = 64
P = 128


def _bcast_row(ap, reps):
    """[1, C] AP -> [1, reps, C] AP with step-0 middle dim."""
    return bass.AP(
        tensor=ap.tensor,
        offset=ap.offset,
        ap=[list(ap.ap[0]), [0, reps], list(ap.ap[1])],
    )


def build(n_rows=N_ROWS, tile_rows=4096):
    """Build the single-core Tile program (SPMD across cores via inputs)."""
    assert n_rows % tile_rows == 0
    nt = n_rows // tile_rows          # number of big tiles
    jb = tile_rows // P               # row-blocks of 128 rows per tile
    kb = jb // 2                      # transpose pairs per tile
    free_f32 = tile_rows * C // P     # f32 elems per partition per tile
    assert kb % 8 == 0
    gb = kb // 8                      # groups of 8 pairs (= one psum bank)

    nc = bacc.Bacc(
        "TRN2", target_bir_lowering=False, debug=False, num_devices=N_CORES
    )
    x_d = nc.dram_tensor("x", [n_rows, C], F32, kind="ExternalInput").ap()
    a_d = nc.dram_tensor("A", [C, C], F32, kind="ExternalInput").ap()
    b_d = nc.dram_tensor("B", [C, C], F32, kind="ExternalInput").ap()
    o_d = nc.dram_tensor("out", [n_rows, C], F32, kind="ExternalOutput").ap()

    with tile.TileContext(nc) as tc:
        with (
            tc.tile_pool(name="consts", bufs=1) as consts,
            tc.tile_pool(name="xin", bufs=3) as xin,
            tc.tile_pool(name="xbfp", bufs=2) as xbfp,
            tc.tile_pool(name="xtp", bufs=2 * nt * gb) as xtp,
            tc.tile_pool(name="outp", bufs=3) as outp,
            tc.tile_pool(name="statsp", bufs=1) as statsp,
            tc.tile_pool(name="scratchp", bufs=2) as scratchp,
            tc.tile_pool(name="tpsum", bufs=2, space="PSUM") as tpsum,
            tc.tile_pool(name="opsum", bufs=4, space="PSUM") as opsum,
            tc.tile_pool(name="spsum", bufs=1, space="PSUM") as spsum,
        ):
            # ---- constants ----
            ident = consts.tile([P, P], BF16)
            make_identity(nc, ident[:])
            # block-diagonal [[A, 0], [0, A]] in bf16
            a_f32 = consts.tile([P, C], F32)
            nc.gpsimd.dma_start(out=a_f32[0:64, :], in_=a_d)
            nc.gpsimd.dma_start(out=a_f32[64:128, :], in_=a_d)
            a2_bf = consts.tile([P, P], BF16)
            nc.vector.memset(a2_bf[:], 0.0)
            nc.vector.tensor_copy(out=a2_bf[0:64, 0:64], in_=a_f32[0:64, :])
            nc.vector.tensor_copy(out=a2_bf[64:128, 64:128], in_=a_f32[64:128, :])
            b_sb = consts.tile([64, C], F32)
            nc.gpsimd.dma_start(out=b_sb[:], in_=b_d)
            ones_p = consts.tile([P, 1], F32)
            nc.vector.memset(ones_p[:], 1.0)
            ones2_bf = consts.tile([2, P], BF16)
            nc.vector.memset(ones2_bf[:], 1.0)

            stats = statsp.tile([P, nt * C], F32)

            # ---- phase 1: load + cast + reduce + pair-transpose ----
            xts = []  # [tile][g] -> resident bf16 x^T tiles [128, 1024]
            for t in range(nt):
                xf = xin.tile([P, free_f32], F32)
                nc.sync.dma_start(
                    out=xf[:],
                    in_=x_d[t * tile_rows : (t + 1) * tile_rows, :].rearrange(
                        "(p j) c -> p (j c)", p=P
                    ),
                )
                xb = xbfp.tile([P, free_f32], BF16)
                nc.vector.tensor_copy(out=xb[:], in_=xf[:])
                # fp32 column sums via folding adds (contiguous, c-aligned)
                sc = scratchp.tile([P, free_f32 // 2], F32)
                half = free_f32 // 2
                nc.vector.tensor_add(
                    out=sc[:, 0:half], in0=xf[:, 0:half], in1=xf[:, half : 2 * half]
                )
                while half > C:
                    half //= 2
                    nc.vector.tensor_add(
                        out=sc[:, 0:half],
                        in0=sc[:, 0:half],
                        in1=sc[:, half : 2 * half],
                    )
                nc.vector.tensor_copy(
                    out=stats[:, t * C : (t + 1) * C], in_=sc[:, 0:C]
                )
                per_tile = []
                for g in range(gb):
                    tb = tpsum.tile([P, 1024], BF16, tag="tb")
                    for u in range(8):
                        k = 8 * g + u
                        nc.tensor.transpose(
                            out=tb[:, 128 * u : 128 * u + 128],
                            in_=xb[:, 128 * k : 128 * k + 128],
                            identity=ident[:],
                        )
                    xt_sb = xtp.tile([P, 1024], BF16, tag="xt")
                    nc.scalar.copy(out=xt_sb[:], in_=tb[:])
                    per_tile.append(xt_sb)
                xts.append(per_tile)

            # ---- interlude: s (fp32) -> -s@B -> bf16 hi/lo rhs rows ----
            half = (nt * C) // 2
            while half >= C:
                nc.vector.tensor_add(
                    out=stats[:, 0:half],
                    in0=stats[:, 0:half],
                    in1=stats[:, half : 2 * half],
                )
                half //= 2
            st_ps = spsum.tile([64, 1], F32, tag="sps")
            nc.tensor.matmul(
                out=st_ps[:], lhsT=stats[:, 0:C], rhs=ones_p[:], start=True, stop=True
            )
            nst_sb = consts.tile([64, 1], F32)
            nc.scalar.copy(out=nst_sb[:], in_=st_ps[:])
            nc.vector.tensor_scalar_mul(out=nst_sb[:], in0=nst_sb[:], scalar1=-1.0)
            nsb_ps = spsum.tile([1, C], F32, tag="sps")
            nc.tensor.matmul(
                out=nsb_ps[:], lhsT=nst_sb[:], rhs=b_sb[:], start=True, stop=True
            )
            nsb_sb = consts.tile([1, C], F32)
            nc.scalar.copy(out=nsb_sb[:], in_=nsb_ps[:])
            hi_bf = consts.tile([1, C], BF16)
            nc.scalar.copy(out=hi_bf[:], in_=nsb_sb[:])
            hi_f32 = consts.tile([1, C], F32)
            nc.scalar.copy(out=hi_f32[:], in_=hi_bf[:])
            lo_f32 = consts.tile([1, C], F32)
            nc.vector.tensor_sub(out=lo_f32[:], in0=nsb_sb[:], in1=hi_f32[:])
            lo_bf = consts.tile([1, C], BF16)
            nc.scalar.copy(out=lo_bf[:], in_=lo_f32[:])
            sbrhs = consts.tile([2, 512], BF16)
            nc.scalar.copy(
                out=sbrhs[0:1, :].rearrange("p (r c) -> p r c", c=C),
                in_=_bcast_row(hi_bf[:], 8),
            )
            # engines cannot write at partition offset 1; stage the lo row and
            # move it with a tiny SBUF->SBUF DMA instead
            lo8 = consts.tile([1, 512], BF16)
            nc.scalar.copy(
                out=lo8[:].rearrange("p (r c) -> p r c", c=C),
                in_=_bcast_row(lo_bf[:], 8),
            )
            nc.gpsimd.dma_start(out=sbrhs[1:2, :], in_=lo8[:])

            # ---- phase 2: matmuls + (-s@B) accumulate + evict + store ----
            for t in range(nt):
                ot = outp.tile([P, free_f32], F32)
                for b in range(2 * gb):
                    ob = opsum.tile([P, 512], F32, tag="ob")
                    for u in range(4):
                        k = 4 * b + u
                        xt_sb = xts[t][k // 8]
                        col = 128 * (k % 8)
                        nc.tensor.matmul(
                            out=ob[:, 128 * u : 128 * u + 128],
                            lhsT=xt_sb[:, col : col + 128],
                            rhs=a2_bf[:],
                            start=(u == 0),
                            stop=False,
                        )
                    nc.tensor.matmul(
                        out=ob[:],
                        lhsT=ones2_bf[:],
                        rhs=sbrhs[:],
                        start=False,
                        stop=True,
                    )
                    nc.scalar.copy(out=ot[:, 512 * b : 512 * b + 512], in_=ob[:])
                nc.sync.dma_start(
                    out=o_d[t * tile_rows : (t + 1) * tile_rows, :].rearrange(
                        "(p j) c -> p (j c)", p=P
                    ),
                    in_=ot[:],
                )

    nc.compile()
    return nc


_CACHE = {}


def _get_compiled():
    if "nc" not in _CACHE:
        _CACHE["nc"] = build()
    return _CACHE["nc"]


def _run(nc, x, A, B, **kwargs):
    from concourse.bass_utils import run_bass_kernel_spmd

    x = np.ascontiguousarray(np.asarray(x, dtype=np.float32))
    A = np.ascontiguousarray(np.asarray(A, dtype=np.float32))
    B = np.ascontiguousarray(np.asarray(B, dtype=np.float32))
    n_cores = x.shape[0]
    in_maps = [{"x": x[i], "A": A, "B": B} for i in range(n_cores)]
    res = run_bass_kernel_spmd(nc, in_maps, core_ids=list(range(n_cores)), **kwargs)
    out = np.stack([res.results[i]["out"] for i in range(n_cores)], axis=0)
    return out, res


def kernel(x, A, B):
    nc = _get_compiled()
    out, _ = _run(nc, x, A, B)
    return out.astype(np.float32)


# revision 7
# speedup vs baseline: 1.4996x; 1.2590x over previous
"""Trainium2 Bass kernel for nn_EquivariantLayer (gnn_message_passing).

Computes, per batch element:  out = x @ A - ones(N,1) @ (colsum(x) @ B)
with x [65536, 64] f32, A/B [64, 64] f32.

Sharding: batch axis (8) -> 8 NeuronCores, A/B replicated; no collectives.

Per-core dataflow (two phases; input and output DMA cannot overlap because
every output row depends on colsum(x)):
  Phase 1 (streaming tiles):
    - HWDGE DMA x tile (contiguous) into SBUF fp32
    - DVE cast fp32 -> bf16; DVE folding adds accumulate fp32 column sums
    - PE pair-transposes [128,128] bf16 blocks into PSUM
    - ACT evicts transposed blocks into resident bf16 x^T tiles (8 MiB)
  Interlude: s = colsum(x) (fp32); -s@B split into bf16 hi+lo rows
  Phase 2 (per tile):
    - PE matmuls: x^T pair block [128,128] stationary, block-diag [[A,0],[0,A]]
      bf16 moving -> natural [128,128] fp32 PSUM (start=True, stop=False)
    - PE K=2 ones-matmul accumulates -(s@B) (hi+lo bf16) onto each PSUM bank
    - ACT plain-copy evicts bank -> out staging
    - HWDGE DMA out tile (contiguous)

Precision: s path and -s@B fully fp32 (hi/lo bf16 split is exact to ~2^-17);
only x@A runs in bf16.  Output norm is dominated by the s@B term, so overall
rel err ~1e-4.
"""

import sys

for _p in ("/opt/trn_rl_repo",):
    if _p not in sys.path:
        sys.path.insert(0, _p)

import numpy as np

import concourse.bass as bass
import concourse.tile as tile
from concourse import bacc, mybir
from concourse.masks import make_identity

F32 = mybir.dt.float32
BF16 = mybir.dt.bfloat16

N_CORES = 8
N_ROWS = 65536
C = 64
P = 128


def _bcast_row(ap, reps):
    """[1, C] AP -> [1, reps, C] AP with step-0 middle dim."""
    return bass.AP(
        tensor=ap.tensor,
        offset=ap.offset,
        ap=[list(ap.ap[0]), [0, reps], list(ap.ap[1])],
    )


def build(n_rows=N_ROWS, tile_rows=4096):
    """Build the single-core Tile program (SPMD across cores via inputs)."""
    assert n_rows % tile_rows == 0
    nt = n_rows // tile_rows          # number of big tiles
    jb = tile_rows // P               # row-blocks of 128 rows per tile
    kb = jb // 2                      # transpose pairs per tile
    free_f32 = tile_rows * C // P     # f32 elems per partition per tile
    assert kb % 8 == 0
    gb = kb // 8                      # groups of 8 pairs (= one psum bank)

    nc = bacc.Bacc(
        "TRN2", target_bir_lowering=False, debug=False, num_devices=N_CORES
    )
    x_d = nc.dram_tensor("x", [n_rows, C], F32, kind="ExternalInput").ap()
    a_d = nc.dram_tensor("A", [C, C], F32, kind="ExternalInput").ap()
    b_d = nc.dram_tensor("B", [C, C], F32, kind="ExternalInput").ap()
    o_d = nc.dram_tensor("out", [n_rows, C], F32, kind="ExternalOutput").ap()

    with tile.TileContext(nc) as tc:
        with (
            tc.tile_pool(name="consts", bufs=1) as consts,
            tc.tile_pool(name="xin", bufs=3) as xin,
            tc.tile_pool(name="xbfp", bufs=2) as xbfp,
            tc.tile_pool(name="xtp", bufs=2 * nt * gb) as xtp,
            tc.tile_pool(name="outp", bufs=3) as outp,
            tc.tile_pool(name="statsp", bufs=1) as statsp,
            tc.tile_pool(name="scratchp", bufs=2) as scratchp,
            tc.tile_pool(name="tpsum", bufs=2, space="PSUM") as tpsum,
            tc.tile_pool(name="opsum", bufs=4, space="PSUM") as opsum,
            tc.tile_pool(name="spsum", bufs=1, space="PSUM") as spsum,
        ):
            # ---- constants ----
            ident = consts.tile([P, P], BF16)
            make_identity(nc, ident[:])
            # block-diagonal [[A, 0], [0, A]] in bf16
            a_f32 = consts.tile([P, C], F32)
            nc.gpsimd.dma_start(out=a_f32[0:64, :], in_=a_d)
            nc.gpsimd.dma_start(out=a_f32[64:128, :], in_=a_d)
            a2_bf = consts.tile([P, P], BF16)
            nc.vector.memset(a2_bf[:], 0.0)
            nc.vector.tensor_copy(out=a2_bf[0:64, 0:64], in_=a_f32[0:64, :])
            nc.vector.tensor_copy(out=a2_bf[64:128, 64:128], in_=a_f32[64:128, :])
            b_sb = consts.tile([64, C], F32)
            nc.gpsimd.dma_start(out=b_sb[:], in_=b_d)
            ones_p = consts.tile([P, 1], F32)
            nc.vector.memset(ones_p[:], 1.0)
            ones_m = consts.tile([64, P], F32)
            nc.vector.memset(ones_m[:], 1.0)

            stats = statsp.tile([P, nt * C], F32)

            # ---- phase 1: load + cast + reduce + pair-transpose ----
            xts = []  # [tile][g] -> resident bf16 x^T tiles [128, 1024]
            for t in range(nt):
                xf = xin.tile([P, free_f32], F32)
                nc.sync.dma_start(
                    out=xf[:],
                    in_=x_d[t * tile_rows : (t + 1) * tile_rows, :].rearrange(
                        "(p j) c -> p (j c)", p=P
                    ),
                )
                xb = xbfp.tile([P, free_f32], BF16)
                # split cast between ACT and DVE to balance phase-1 load
                if t % 2 == 0:
                    nc.scalar.copy(out=xb[:], in_=xf[:])
                else:
                    nc.vector.tensor_copy(out=xb[:], in_=xf[:])
                # fp32 column sums via folding adds (contiguous, c-aligned)
                sc = scratchp.tile([P, free_f32 // 2], F32)
                half = free_f32 // 2
                nc.vector.tensor_add(
                    out=sc[:, 0:half], in0=xf[:, 0:half], in1=xf[:, half : 2 * half]
                )
                while half > 2 * C:
                    half //= 2
                    nc.vector.tensor_add(
                        out=sc[:, 0:half],
                        in0=sc[:, 0:half],
                        in1=sc[:, half : 2 * half],
                    )
                nc.vector.tensor_add(
                    out=stats[:, t * C : (t + 1) * C],
                    in0=sc[:, 0:C],
                    in1=sc[:, C : 2 * C],
                )
                per_tile = []
                for g in range(gb):
                    tb = tpsum.tile([P, 1024], BF16, tag="tb")
                    for u in range(8):
                        k = 8 * g + u
                        nc.tensor.transpose(
                            out=tb[:, 128 * u : 128 * u + 128],
                            in_=xb[:, 128 * k : 128 * k + 128],
                            identity=ident[:],
                        )
                    xt_sb = xtp.tile([P, 1024], BF16, tag="xt")
                    nc.scalar.copy(out=xt_sb[:], in_=tb[:])
                    per_tile.append(xt_sb)
                xts.append(per_tile)

            # ---- interlude: s (fp32) -> -s@B -> bf16 hi/lo rhs rows ----
            half = (nt * C) // 2
            while half >= C:
                nc.vector.tensor_add(
                    out=stats[:, 0:half],
                    in0=stats[:, 0:half],
                    in1=stats[:, half : 2 * half],
                )
                half //= 2
            st_ps = spsum.tile([64, 1], F32, tag="sps")
            nc.tensor.matmul(
                out=st_ps[:], lhsT=stats[:, 0:C], rhs=ones_p[:], start=True, stop=True
            )
            st_sb = consts.tile([64, 1], F32)
            nc.scalar.copy(out=st_sb[:], in_=st_ps[:])
            # bs = B * s (per-partition scale); bc = ones (x) (s@B)  [128, 64]
            bs_sb = consts.tile([64, C], F32)
            nc.vector.tensor_scalar_mul(out=bs_sb[:], in0=b_sb[:], scalar1=st_sb[:])
            bc_ps = spsum.tile([P, C], F32, tag="sps")
            nc.tensor.matmul(
                out=bc_ps[:], lhsT=ones_m[:], rhs=bs_sb[:], start=True, stop=True
            )
            bc_sb = consts.tile([P, C], F32)
            nc.scalar.copy(out=bc_sb[:], in_=bc_ps[:])
            bc_bcast = _bcast_row(bc_sb[:], 8)

            # ---- phase 2: matmuls + subtract-evict + store ----
            for t in range(nt):
                ot = outp.tile([P, free_f32], F32)
                for b in range(2 * gb):
                    ob = opsum.tile([P, 512], F32, tag="ob")
                    for u in range(4):
                        k = 4 * b + u
                        xt_sb = xts[t][k // 8]
                        col = 128 * (k % 8)
                        nc.tensor.matmul(
                            out=ob[:, 128 * u : 128 * u + 128],
                            lhsT=xt_sb[:, col : col + 128],
                            rhs=a2_bf[:],
                            start=(u == 0),
                            stop=(u == 3),
                        )
                    nc.vector.tensor_sub(
                        out=ot[:, 512 * b : 512 * b + 512].rearrange(
                            "p (j c) -> p j c", c=C
                        ),
                        in0=ob[:].rearrange("p (j c) -> p j c", c=C),
                        in1=bc_bcast,
                    )
                nc.sync.dma_start(
                    out=o_d[t * tile_rows : (t + 1) * tile_rows, :].rearrange(
                        "(p j) c -> p (j c)", p=P
                    ),
                    in_=ot[:],
                )

    nc.compile()
    return nc


_CACHE = {}


def _get_compiled():
    if "nc" not in _CACHE:
        _CACHE["nc"] = build()
    return _CACHE["nc"]


def _run(nc, x, A, B, **kwargs):
    from concourse.bass_utils import run_bass_kernel_spmd

    x = np.ascontiguousarray(np.asarray(x, dtype=np.float32))
    A = np.ascontiguousarray(np.asarray(A, dtype=np.float32))
    B = np.ascontiguousarray(np.asarray(B, dtype=np.float32))
    n_cores = x.shape[0]
    in_maps = [{"x": x[i], "A": A, "B": B} for i in range(n_cores)]
    res = run_bass_kernel_spmd(nc, in_maps, core_ids=list(range(n_cores)), **kwargs)
    out = np.stack([res.results[i]["out"] for i in range(n_cores)], axis=0)
    return out, res


def kernel(x, A, B):
    nc = _get_compiled()
    out, _ = _run(nc, x, A, B)
    return out.astype(np.float32)
